# revision 26
# baseline (speedup 1.0000x reference)
"""MoE feed-forward (dense-routing reference) on 8 trn2 NeuronCores.

Strategy: expert-parallel, one expert per core, exploiting top-2 sparsity —
only tokens routed to an expert are sent to its core (~T*K/E + padding
instead of T tokens per expert, a 4x FLOP cut vs dense all-expert compute).

Host side (numpy): gating (fp64 logits -> softmax -> top-2; the fp64
ordering agrees with any fp32 backend's ordering unless a token's 2nd/3rd
logit gap is ~1e-6 — verified safe for this input distribution), gather of
each expert's tokens into a transposed bf16 [H, C] batch, and scatter-add
of the per-expert outputs. Weights are pre-cast to bf16 on host so the
device DMAs them straight into resident SBUF tiles (no staging/casts).

Device side (Bass/Tile, per core), all matmuls bf16 with fp32 PSUM:
  hiddenT[f, tok] = relu(w1[h, f].T @ xT[h, tok] + b1[f])
  y[tok, h]      = (hiddenT[f, tok].T @ w2[f, h]) * gate[tok]
Weights stay resident in SBUF; tokens stream in chunks of <=512.
"""

import hashlib
import os
import time
from concurrent.futures import ThreadPoolExecutor

import ml_dtypes
import numpy as np

import concourse.tile as tile
from concourse import bacc, mybir
from concourse.bass_utils import run_bass_kernel_spmd

BF16 = ml_dtypes.bfloat16
P = 128
H = 1024
F = 4096
E = 8
TOPK = 2
HT = H // P   # 8 contraction tiles for MM1
FT = F // P   # 32 contraction tiles for MM2
TOK = 512     # token chunk (moving-dim N for MM1)
NH = 512      # output column tile for MM2

PROFILE = False          # test harness sets True to try to get a trace
LAST_RESULT = None       # BassKernelResults of last run

_nc_cache = {}


def _chunks(C):
    # Token chunks of 512, avoiding a short 128 tail (moving dims >= 256
    # keep the PE streaming efficient): a remainder of 640 splits 384+256.
    out = []
    c0 = 0
    while c0 < C:
        rem = C - c0
        t = min(TOK, rem) if rem != 640 else 384
        out.append((c0, t))
        c0 += t
    return out


def _build(C):
    """Build + compile the per-core SPMD program for token capacity C."""
    assert C % P == 0
    f32 = mybir.dt.float32
    bf16 = mybir.dt.bfloat16

    nc = bacc.Bacc("TRN2", target_bir_lowering=False, debug=False)
    xT_d = nc.dram_tensor("xT", [H, C], bf16, kind="ExternalInput")
    w1_d = nc.dram_tensor("w1", [H, F], bf16, kind="ExternalInput")
    b1_d = nc.dram_tensor("b1", [P, FT], f32, kind="ExternalInput")
    w2_d = nc.dram_tensor("w2", [F, H], bf16, kind="ExternalInput")
    sc_d = nc.dram_tensor("sc", [P, C // P], f32, kind="ExternalInput")
    y_d = nc.dram_tensor("y", [C, H], f32, kind="ExternalOutput")

    with tile.TileContext(nc) as tc:
        with (
            tc.tile_pool(name="wres", bufs=1) as wres,
            tc.tile_pool(name="consts", bufs=1) as consts,
            tc.tile_pool(name="xpool", bufs=2) as xpool,
            tc.tile_pool(name="hidp", bufs=1) as hidp,
            tc.tile_pool(name="outp", bufs=3) as outp,
            tc.tile_pool(name="psum1", bufs=4, space="PSUM") as psum1,
            tc.tile_pool(name="psum2", bufs=4, space="PSUM") as psum2,
        ):
            w1_sb = wres.tile([P, HT, F], bf16)      # 64KB/partition
            w2_sb = wres.tile([P, FT, H], bf16)      # 64KB/partition
            b1_sb = consts.tile([P, FT], f32)
            sc_sb = consts.tile([P, C // P], f32)

            # Resident weights, straight from DRAM (already bf16).
            # w1 lands f-major (1024-wide column pieces) so MM1 of the first
            # chunk can start as soon as the first pieces arrive. w2 isn't
            # needed until chunk-0 MM2, so it's emitted (= prioritized)
            # after chunk-0's MM1 work, below.
            for q in range(4):
                for h in range(HT):
                    nc.sync.dma_start(
                        out=w1_sb[:, h, q * 1024:(q + 1) * 1024],
                        in_=w1_d[h * P:(h + 1) * P, q * 1024:(q + 1) * 1024],
                    )

            first_chunk = True
            x_engs = [nc.gpsimd, nc.scalar]
            for ci, (c0, t) in enumerate(_chunks(C)):
                xb = xpool.tile([P, HT, TOK], bf16, tag="xb")
                hid_sb = hidp.tile([P, FT, TOK], bf16, tag="hid")
                for h in range(HT):
                    x_engs[ci % 2].dma_start(
                        out=xb[:, h, :t], in_=xT_d[h * P:(h + 1) * P, c0:c0 + t]
                    )
                if ci == 0:
                    # b1/sc aren't needed until the first eviction; loading
                    # them after chunk-0's x keeps x off the SWDGE ring head.
                    nc.gpsimd.dma_start(out=b1_sb, in_=b1_d[:, :])
                    nc.gpsimd.dma_start(out=sc_sb, in_=sc_d[:, :])

                # MM1: hiddenT[f, tok] = relu(w1.T @ xT + b1)
                for i in range(FT):
                    ps = psum1.tile([P, TOK], f32, tag="ps1")
                    for h in range(HT):
                        nc.tensor.matmul(
                            ps[:, :t],
                            w1_sb[:, h, i * P:(i + 1) * P],
                            xb[:, h, :t],
                            start=(h == 0),
                            stop=(h == HT - 1),
                        )
                    nc.scalar.activation(
                        hid_sb[:, i, :t],
                        ps[:, :t],
                        mybir.ActivationFunctionType.Relu,
                        bias=b1_sb[:, i:i + 1],
                    )

                if first_chunk:
                    first_chunk = False
                    for i in range(FT):
                        nc.gpsimd.dma_start(
                            out=w2_sb[:, i, :], in_=w2_d[i * P:(i + 1) * P, :]
                        )

                # MM2: y[tok, h] = (hiddenT.T @ w2) * gate
                for m in range(t // P):
                    mg = c0 // P + m
                    for nh in range(H // NH):
                        ps = psum2.tile([P, NH], f32, tag="ps2")
                        for i in range(FT):
                            nc.tensor.matmul(
                                ps,
                                hid_sb[:, i, m * P:(m + 1) * P],
                                w2_sb[:, i, nh * NH:(nh + 1) * NH],
                                start=(i == 0),
                                stop=(i == FT - 1),
                            )
                        ot = outp.tile([P, NH], f32, tag="out")
                        nc.vector.tensor_scalar_mul(ot, ps, sc_sb[:, mg:mg + 1])
                        nc.sync.dma_start(
                            out=y_d[c0 + m * P:c0 + (m + 1) * P,
                                    nh * NH:(nh + 1) * NH],
                            in_=ot,
                        )

    nc.compile()
    return nc


def _get_nc(C):
    if C not in _nc_cache:
        _nc_cache[C] = _build(C)
    return _nc_cache[C]


def _install_neff_disk_cache():
    """Wrap bass2jax's neuronx_cc hook with a content-hash NEFF disk cache.

    The BIR -> NEFF compile (walrus) takes ~2 min and bass2jax does not
    cache it across processes; the HLO payload is deterministic for a
    given program, so a sha256-keyed file cache makes repeat runs fast.
    """
    from concourse import bass2jax as b2j

    if getattr(b2j, "_moe_neff_cache_installed", False):
        return
    base = b2j.neuronx_cc_hook
    cache_dir = os.path.join(
        os.environ.get("XDG_CACHE_HOME", os.path.expanduser("~/.cache")),
        "moe-bass-neff",
    )

    def _scrub_debug(obj):
        # Debug records (source path / lineno / traceback) appear both under
        # "ant_debug" keys and as bare records in lists; drop them all so the
        # key is independent of where kernel.py was imported from.
        if isinstance(obj, dict):
            if "ant_traceback" in obj or ("filename" in obj and "lineno" in obj):
                return {}
            return {
                k: _scrub_debug(v) for k, v in obj.items() if k != "ant_debug"
            }
        if isinstance(obj, list):
            return [_scrub_debug(v) for v in obj]
        return obj

    def _cache_key(code):
        """sha256 of the HLO module with jit-counter and source-location
        noise removed, so the key is stable across processes and across
        the directory kernel.py is imported from."""
        try:
            import json

            import libneuronxla.proto.hlo_pb2 as hlo_pb2

            proto = hlo_pb2.HloModuleProto.FromString(bytes(code))
            proto.id = 0
            proto.name = "m"
            if proto.HasField("stack_frame_index"):
                proto.ClearField("stack_frame_index")
            for comp in proto.computations:
                for ins in comp.instructions:
                    ins.ClearField("metadata")
                    if (
                        ins.opcode == "custom-call"
                        and ins.custom_call_target == "bass_exec"
                    ):
                        cfg = json.loads(
                            __import__("base64").standard_b64decode(
                                ins.backend_config
                            )
                        )
                        bir = json.loads(b2j._decompress_ant_bir(cfg["ant_bir"]))
                        cfg["ant_bir"] = hashlib.sha256(
                            json.dumps(
                                _scrub_debug(bir), sort_keys=True
                            ).encode()
                        ).hexdigest()
                        ins.backend_config = json.dumps(
                            cfg, sort_keys=True
                        ).encode()
            blob = proto.SerializeToString(deterministic=True)
        except Exception:
            blob = bytes(code)
        return hashlib.sha256(blob).hexdigest()

    def cached_hook(code, code_format, platform_version, file_prefix):
        if b"bass_exec" not in code:
            return base(code, code_format, platform_version, file_prefix)
        try:
            os.makedirs(cache_dir, exist_ok=True)
            key = _cache_key(code)
            path = os.path.join(cache_dir, key + ".bin")
            if os.path.exists(path):
                with open(path, "rb") as f:
                    return 0, f.read()
        except Exception:
            path = None
        ret = base(code, code_format, platform_version, file_prefix)
        try:
            if path is not None and ret[0] == 0 and isinstance(ret[1], bytes):
                tmp = f"{path}.tmp{os.getpid()}"
                with open(tmp, "wb") as f:
                    f.write(ret[1])
                os.replace(tmp, path)
        except Exception:
            pass
        return ret

    b2j.neuronx_cc_hook = cached_hook
    b2j._moe_neff_cache_installed = True


_install_neff_disk_cache()


def kernel(x, gate_w, w1, b1, w2, b2):
    global LAST_RESULT
    S, B, h_in = x.shape
    assert h_in == H and gate_w.shape == (E, H), (x.shape, gate_w.shape)
    assert w1.shape == (E, H, F) and w2.shape == (E, F, H), (w1.shape, w2.shape)
    T = S * B
    xf = np.ascontiguousarray(x.reshape(T, H), dtype=np.float32)

    # --- host gating ---
    # fp32 sgemm for speed; rows whose 2nd/3rd logit gap is small get
    # recomputed in fp64 so the top-2 ordering is backend-stable (fp32
    # noise is ~3e-6, far under the 1e-3 refinement threshold).
    gw32 = np.asarray(gate_w, dtype=np.float32)
    l32 = xf @ gw32.T
    logits = l32.astype(np.float64)
    ls = np.sort(l32, axis=-1)
    risky = np.nonzero(ls[:, -2] - ls[:, -3] < 1e-3)[0]
    if len(risky):
        logits[risky] = xf[risky].astype(np.float64) @ gw32.T.astype(np.float64)
    top2 = np.argsort(-logits, axis=-1)[:, :TOPK]
    lm = logits - logits.max(axis=-1, keepdims=True)
    p = np.exp(lm)
    probs = (p / p.sum(axis=-1, keepdims=True)).astype(np.float32)

    xfT = np.ascontiguousarray(xf.T.astype(BF16))  # [H, T] bf16
    onehot = np.zeros((T, E), dtype=bool)
    onehot[np.arange(T)[:, None], top2] = True
    sel = [np.nonzero(onehot[:, e])[0] for e in range(E)]

    Cmax = max(len(s) for s in sel)
    C = max(((Cmax + P - 1) // P) * P, 2 * P)
    nc = _get_nc(C)

    def _make_in_map(e):
        se = sel[e]
        n = len(se)
        xT_e = np.zeros((H, C), BF16)
        np.take(xfT, se, axis=1, out=xT_e[:, :n])
        sc_e = np.zeros((C,), np.float32)
        sc_e[:n] = probs[se, e]
        return {
            "xT": xT_e,
            "w1": np.asarray(w1[e], dtype=np.float32).astype(BF16),
            "b1": np.ascontiguousarray(
                np.asarray(b1[e], dtype=np.float32).reshape(FT, P).T),
            "w2": np.asarray(w2[e], dtype=np.float32).astype(BF16),
            "sc": np.ascontiguousarray(sc_e.reshape(C // P, P).T),
        }

    with ThreadPoolExecutor(E) as pool:
        in_maps = list(pool.map(_make_in_map, range(E)))

    r = None
    if PROFILE:
        try:
            r = run_bass_kernel_spmd(nc, in_maps, list(range(E)), trace=True)
        except Exception:
            r = None
    if r is None:
        # A transiently wedged NeuronCore (NRT_EXEC_UNIT_UNRECOVERABLE)
        # recovers on re-dispatch; retry before giving up.
        for attempt in range(3):
            try:
                r = run_bass_kernel_spmd(nc, in_maps, list(range(E)))
                break
            except Exception:
                if attempt == 2:
                    raise
                time.sleep(2.0)
    LAST_RESULT = r

    y = np.zeros((T, H), np.float32)
    for e in range(E):
        se = sel[e]
        y[se] += r.results[e]["y"][:len(se)]
    if np.any(b2):
        W = np.zeros((T, E), np.float32)
        W[np.arange(T)[:, None], top2] = probs[np.arange(T)[:, None], top2]
        y += W @ np.asarray(b2, dtype=np.float32)
    return y.reshape(S, B, H)
